# revision 33
# baseline (speedup 1.0000x reference)
"""Trainium2 Bass kernel for AdaptiveNoisingModule (retrieval kNN).

Math (matches the jax reference):
  f = features (B,C,H,W) -> (B*H*W, C) query rows
  d2[n,m] = |f_n|^2 + |mb_m|^2 - 2 f_n.mb_m ; nearest = argmin_m d2
  influence[n] = mean_c |f_n - mb_nearest| / (sqrt(clip(d2min,0)+1e-8) + 1e-8)
  influence_norm = (influence - min) / (max - min)   [global min/max]
  noise_std = 0.01 + 0.49 * influence_norm
  noised = f + noise * noise_std      (noise = jax.random.normal(key(1)))

Sharding: data-parallel over queries; batch image i -> core i (784 rows each).
Memory bank replicated. Only the global min/max of influence is all-reduced.

Device algorithm per core:
  score[n,m] = 2 f.mb - |mb|^2: f32r matmul (full TensorE rate; operands
  rounded to 11 mantissa bits by HW) accumulated over 8 k-blocks in PSUM;
  |mb|^2 subtracted in f32 on VectorE during the PSUM->SBUF move.
  argmin_m d2 = argmax_m score: DVE MAX8/MAX_INDEX over 1024-wide m-groups,
  tiny cross-group combine (first-index tie-break like jax top_k). Nearest
  rows fetched with an indirect (gather) DMA; influence on DVE/ACT; the
  per-block post-processing is interleaved into the last m-group's loop so
  it hides under the remaining matmuls. Global min/max via a packed [1,2]
  AllReduce(max) of (max, -min); noising fused with scalar_tensor_tensor.
"""

import numpy as np

B, C, H, W = 8, 1024, 28, 28
M = 16384
N_CORES = 8
NQ = H * W          # 784 queries per core (batch-sharded)
CE = 1024           # contraction dim (= C)
KB = CE // 128      # 8 k-blocks
G = 16              # m-groups
GW = M // G         # 1024 group width
NT = GW // 512      # psum tiles per group
P = 128
NB = (NQ + P - 1) // P          # 7 row blocks (6x128 + 16)
BLK = [(i * P, min(P, NQ - i * P)) for i in range(NB)]
EPS = 1e-8
NOISE_MIN, NOISE_MAX = 0.01, 0.5
BIG = 3.0e7

_CACHE = {}


def build_bass(stage=4):
    """Build the SPMD Bass graph (same program for all 8 cores).

    stage: 3 = core-local min/max (debug), 4 = full (8-core AllReduce).
    """
    if ("nc", stage) in _CACHE:
        return _CACHE[("nc", stage)]
    from contextlib import ExitStack
    import concourse.bass as bass
    import concourse.tile as tile
    from concourse import bacc, mybir

    f32 = mybir.dt.float32
    f32r = mybir.dt.float32r
    u32 = mybir.dt.uint32
    AX = mybir.AxisListType
    OP = mybir.AluOpType
    ACTF = mybir.ActivationFunctionType

    nc = bacc.Bacc("TRN2", target_bir_lowering=False, debug=False,
                   num_devices=N_CORES)

    ft = nc.declare_dram_parameter("ft", [CE, NQ], f32r, isOutput=False)
    mbt = nc.declare_dram_parameter("mbt", [CE, M], f32r, isOutput=False)
    mnorm = nc.declare_dram_parameter("mnorm", [1, M], f32, isOutput=False)
    fq = nc.declare_dram_parameter("fq", [NQ, C], f32, isOutput=False)
    mbrows = nc.declare_dram_parameter("mbrows", [M, C], f32, isOutput=False)
    noise = nc.declare_dram_parameter("noise", [NQ, C], f32, isOutput=False)
    out_x = nc.declare_dram_parameter("out_noised", [NQ, C], f32, isOutput=True)
    out_i = nc.declare_dram_parameter("out_inf", [NQ, 1], f32, isOutput=True)
    out_s = nc.declare_dram_parameter("out_std", [NQ, 1], f32, isOutput=True)

    with tile.TileContext(nc) as tc, ExitStack() as ctx:
        const = ctx.enter_context(tc.tile_pool(name="const", bufs=1))
        mbtp = ctx.enter_context(tc.tile_pool(name="mbtp", bufs=2))
        scp = ctx.enter_context(tc.tile_pool(name="scores", bufs=4))
        psp = ctx.enter_context(tc.tile_pool(name="psum", bufs=4, space="PSUM"))
        wk = ctx.enter_context(tc.tile_pool(name="work", bufs=2))
        sm = ctx.enter_context(tc.tile_pool(name="small", bufs=2))
        drp = ctx.enter_context(tc.tile_pool(name="dram", bufs=1, space="DRAM"))

        # ---- resident tiles ----
        ft_sb = const.tile([P, KB, NQ], f32r)
        nc.sync.dma_start(ft_sb[:], ft.rearrange("(kb p) n -> p kb n", p=P))

        gval = const.tile([P, NB, G, 8], f32)   # top-8 scores per (block, group)
        gidx = const.tile([P, NB, G, 8], u32)   # their in-group indices

        goff = const.tile([P, G], f32)          # column g -> g*GW
        for g in range(G):
            nc.vector.memset(goff[:, g:g + 1], float(g * GW))
        bigc = const.tile([P, G], f32)
        nc.vector.memset(bigc[:], BIG)
        epsc = const.tile([P, 1], f32)
        nc.vector.memset(epsc[:], EPS)

        inf_cols = const.tile([P, NB], f32)     # influence per block column
        infmin = const.tile([P, NB], f32)       # +BIG filled (for min)
        infmax = const.tile([P, NB], f32)       # -BIG filled (for max)
        nc.vector.memset(infmin[:], BIG)
        nc.vector.memset(infmax[:], -BIG)

        # queries + noise resident: prefetched while matmuls run
        fqr = const.tile([P, NB, C], f32)
        nzr = const.tile([P, NB, C], f32)
        qn_cols = const.tile([P, NB], f32)      # |f|^2 per block column
        scr = const.tile([P, C], f32)           # Square scratch (never read)
        for b, (n0, pb) in enumerate(BLK):
            nc.sync.dma_start(fqr[:pb, b, :], fq[n0:n0 + pb, :])
            nc.sync.dma_start(nzr[:pb, b, :], noise[n0:n0 + pb, :])
        for b, (n0, pb) in enumerate(BLK):
            nc.scalar.activation(scr[:pb], fqr[:pb, b, :], ACTF.Square,
                                 accum_out=qn_cols[:pb, b:b + 1])

        def emit_block_post(b, n0, pb):
            """Combine groups -> global argmax -> gather -> influence."""
            vals = gval[:pb, b, :, 0]            # [pb, G] stride-8
            maxv = sm.tile([P, 1], f32, name=f"maxv{b}", tag="maxv")
            nc.vector.tensor_reduce(maxv[:pb], vals, axis=AX.X, op=OP.max)
            mask = sm.tile([P, G], u32, name=f"mask{b}", tag="mask")
            nc.vector.tensor_scalar(mask[:pb], vals, maxv[:pb], None,
                                    op0=OP.is_ge)
            idxf = sm.tile([P, G], f32, name=f"idxf{b}", tag="idxf")
            nc.vector.tensor_copy(idxf[:pb], gidx[:pb, b, :, 0])  # u32 -> f32
            nc.vector.tensor_tensor(idxf[:pb], idxf[:pb], goff[:pb], op=OP.add)
            cand = sm.tile([P, G], f32, name=f"cand{b}", tag="cand")
            nc.vector.select(cand[:pb], mask[:pb], idxf[:pb], bigc[:pb])
            idxm = sm.tile([P, 1], f32, name=f"idxm{b}", tag="idxm")
            nc.vector.tensor_reduce(idxm[:pb], cand[:pb], axis=AX.X, op=OP.min)
            idxu = sm.tile([P, 1], u32, name=f"idxu{b}", tag="idxu")
            nc.vector.tensor_copy(idxu[:pb], idxm[:pb])           # f32 -> u32

            nn = wk.tile([P, C], f32, name=f"nn{b}", tag="nn")
            nc.gpsimd.indirect_dma_start(
                nn[:pb], None, mbrows[:],
                bass.IndirectOffsetOnAxis(ap=idxu[:pb], axis=0),
            )
            fqb = fqr[:, b, :]
            diff = wk.tile([P, C], f32, name=f"diff{b}", tag="diff")
            nc.vector.tensor_tensor(diff[:pb], fqb[:pb], nn[:pb],
                                    op=OP.subtract)
            asum = sm.tile([P, 1], f32, name=f"asum{b}", tag="asum")
            nc.vector.tensor_reduce(asum[:pb], diff[:pb], axis=AX.X, op=OP.add,
                                    apply_absolute_value=True)
            d2 = sm.tile([P, 1], f32, name=f"d2{b}", tag="d2")
            nc.vector.tensor_tensor(d2[:pb], qn_cols[:pb, b:b + 1], maxv[:pb],
                                    op=OP.subtract)
            nc.vector.tensor_scalar_max(d2[:pb], d2[:pb], 0.0)
            dist = sm.tile([P, 1], f32, name=f"dist{b}", tag="dist")
            nc.scalar.activation(dist[:pb], d2[:pb], ACTF.Sqrt, bias=epsc[:pb])
            den = sm.tile([P, 1], f32, name=f"den{b}", tag="den")
            nc.vector.tensor_scalar_add(den[:pb], dist[:pb], EPS)
            rden = sm.tile([P, 1], f32, name=f"rden{b}", tag="rden")
            nc.vector.reciprocal(rden[:pb], den[:pb])
            infb = sm.tile([P, 1], f32, name=f"infb{b}", tag="infb")
            nc.vector.tensor_tensor(infb[:pb], asum[:pb], rden[:pb],
                                    op=OP.mult)
            nc.vector.tensor_scalar_mul(infb[:pb], infb[:pb], 1.0 / C)
            nc.vector.tensor_copy(inf_cols[:pb, b:b + 1], infb[:pb])
            nc.vector.tensor_copy(infmin[:pb, b:b + 1], infb[:pb])
            nc.vector.tensor_copy(infmax[:pb, b:b + 1], infb[:pb])

        # ---- main loop: scores + per-group argmax ----
        # Block post-processing is emitted right after each block's LAST
        # group so it overlaps with the remaining blocks' matmuls.
        for g in range(G):
            mbt_g = mbtp.tile([P, KB, GW], f32r)
            nc.sync.dma_start(
                mbt_g[:],
                mbt[:, g * GW:(g + 1) * GW].rearrange("(kb p) m -> p kb m", p=P),
            )
            mnb = mbtp.tile([P, GW], f32, tag="mnb")
            nc.sync.dma_start(
                mnb[:], mnorm[0:1, g * GW:(g + 1) * GW].partition_broadcast(P)
            )
            for b, (n0, pb) in enumerate(BLK):
                sc = scp.tile([P, GW], f32)
                ps = psp.tile([P, GW], f32)     # two PSUM banks
                for t in range(NT):
                    for kb in range(KB):
                        nc.tensor.matmul(
                            ps[:pb, t * 512:(t + 1) * 512],
                            ft_sb[:, kb, n0:n0 + pb],
                            mbt_g[:, kb, t * 512:(t + 1) * 512],
                            start=(kb == 0),
                            stop=(kb == KB - 1),
                        )
                # score = 2 f.mb - mnorm, fused with the PSUM->SBUF move
                nc.vector.tensor_tensor(sc[:pb], ps[:pb], mnb[:pb],
                                        op=OP.subtract)
                nc.vector.max(gval[:pb, b, g, :], sc[:pb, :])
                nc.vector.max_index(gidx[:pb, b, g, :], gval[:pb, b, g, :],
                                    sc[:pb, :])
                if g == G - 1:
                    emit_block_post(b, n0, pb)

        # ---- global min/max: partition-reduce via DRAM, AllReduce ----
        vmax = sm.tile([P, 1], f32)
        nc.vector.tensor_reduce(vmax[:], infmax[:], axis=AX.X, op=OP.max)
        vmin = sm.tile([P, 1], f32)
        nc.vector.tensor_reduce(vmin[:], infmin[:], axis=AX.X, op=OP.min)
        pk = sm.tile([P, 2], f32)
        nc.vector.tensor_copy(pk[:, 0:1], vmax[:])
        nc.vector.tensor_scalar_mul(pk[:, 1:2], vmin[:], -1.0)  # -min

        dr_pack = drp.tile([1, 2 * P], f32)
        nc.sync.dma_start(dr_pack[:].rearrange("o (p c) -> (o p) c", c=2),
                          pk[:])
        flat = sm.tile([1, 2 * P], f32)
        nc.sync.dma_start(flat[:], dr_pack[:])
        red2 = sm.tile([1, 2], f32)
        nc.vector.tensor_reduce(
            red2[:], flat[0:1].rearrange("o (p c) -> o c p", c=2),
            axis=AX.X, op=OP.max,
        )
        g2 = sm.tile([P, 2], f32)
        if stage >= 4:
            cc_in = drp.tile([1, 2], f32)
            nc.sync.dma_start(cc_in[:], red2[:])
            cc_out = drp.tile([1, 2], f32, addr_space="Shared")
            nc.gpsimd.collective_compute(
                "AllReduce", OP.max,
                replica_groups=[list(range(N_CORES))],
                ins=[cc_in[:].opt()],
                outs=[cc_out[:].opt()],
            )
            nc.sync.dma_start(g2[:], cc_out[:].partition_broadcast(P))
        else:
            # local-only min/max (debug)
            dr2 = drp.tile([1, 2], f32)
            nc.sync.dma_start(dr2[:], red2[:])
            nc.sync.dma_start(g2[:], dr2[:].partition_broadcast(P))

        # scale = 1/(max-min) when max-min > EPS else 0
        delta = sm.tile([P, 1], f32)
        nc.vector.tensor_tensor(delta[:], g2[:, 0:1], g2[:, 1:2], op=OP.add)
        maskd = sm.tile([P, 1], f32)
        nc.vector.tensor_scalar(maskd[:], delta[:], EPS, None, op0=OP.is_gt)
        deltac = sm.tile([P, 1], f32)
        nc.vector.tensor_scalar_max(deltac[:], delta[:], EPS)
        sca = sm.tile([P, 1], f32)
        nc.vector.reciprocal(sca[:], deltac[:])
        nc.vector.tensor_tensor(sca[:], sca[:], maskd[:], op=OP.mult)

        # ---- normalize, noise-scale, output ----
        for b, (n0, pb) in enumerate(BLK):
            normb = sm.tile([P, 1], f32, name=f"normb{b}", tag="normb")
            # (inf + (-min)) * scale   [g2[:,1:2] holds -min]
            nc.vector.scalar_tensor_tensor(
                normb[:pb], inf_cols[:pb, b:b + 1], g2[:pb, 1:2], sca[:pb],
                op0=OP.add, op1=OP.mult,
            )
            stdb = sm.tile([P, 1], f32, name=f"stdb{b}", tag="stdb")
            nc.vector.tensor_scalar(stdb[:pb], normb[:pb],
                                    NOISE_MAX - NOISE_MIN, NOISE_MIN,
                                    op0=OP.mult, op1=OP.add)
            ox = wk.tile([P, C], f32, name=f"ox{b}", tag="ox")
            # noise*std + f
            nc.vector.scalar_tensor_tensor(
                ox[:pb], nzr[:, b, :][:pb], stdb[:pb], fqr[:, b, :][:pb],
                op0=OP.mult, op1=OP.add,
            )
            nc.sync.dma_start(out_x[n0:n0 + pb, :], ox[:pb])
            nc.sync.dma_start(out_i[n0:n0 + pb, :], normb[:pb])
            nc.sync.dma_start(out_s[n0:n0 + pb, :], stdb[:pb])

    nc.compile()
    _CACHE[("nc", stage)] = nc
    return nc


def prepare_in_maps(features, memory_bank, noise_all):
    features = np.ascontiguousarray(features, dtype=np.float32)
    memory_bank = np.ascontiguousarray(memory_bank, dtype=np.float32)
    f_t = features.reshape(B, C, NQ)            # per-image f^T (C, HW)

    mbt_np = np.ascontiguousarray(memory_bank.T)
    mnorm_np = (memory_bank * memory_bank).sum(axis=1, dtype=np.float32)[None, :]

    in_maps = []
    for c in range(N_CORES):
        in_maps.append({
            "ft": np.ascontiguousarray(2.0 * f_t[c]),
            "mbt": mbt_np,
            "mnorm": mnorm_np,
            "fq": np.ascontiguousarray(f_t[c].T),
            "mbrows": memory_bank,
            "noise": np.ascontiguousarray(noise_all[c * NQ:(c + 1) * NQ]),
        })
    return in_maps


def assemble_outputs(results):
    noised = np.empty((B, C, H, W), dtype=np.float32)
    inf_norm = np.empty((B, H, W), dtype=np.float32)
    noise_std = np.empty((B, H, W), dtype=np.float32)
    for c in range(N_CORES):
        r = results[c]
        noised[c] = r["out_noised"].reshape(H, W, C).transpose(2, 0, 1)
        inf_norm[c] = r["out_inf"].reshape(H, W)
        noise_std[c] = r["out_std"].reshape(H, W)
    return noised, inf_norm, noise_std


def make_noise():
    """Same expression as the reference; same process/config -> same values."""
    import jax
    import jax.numpy as jnp
    return np.asarray(
        jax.random.normal(jax.random.key(1), (B * H * W, C), dtype=jnp.float32)
    )


def kernel(features, memory_bank):
    from concourse.bass_utils import run_bass_kernel_spmd

    nc = build_bass()
    in_maps = prepare_in_maps(features, memory_bank, make_noise())
    res = run_bass_kernel_spmd(nc, in_maps, list(range(N_CORES)))
    return assemble_outputs(res.results)


# revision 36
# speedup vs baseline: 1.0437x; 1.0437x over previous
"""Trainium2 Bass kernel for AdaptiveNoisingModule (retrieval kNN).

Math (matches the jax reference):
  f = features (B,C,H,W) -> (B*H*W, C) query rows
  d2[n,m] = |f_n|^2 + |mb_m|^2 - 2 f_n.mb_m ; nearest = argmin_m d2
  influence[n] = mean_c |f_n - mb_nearest| / (sqrt(clip(d2min,0)+1e-8) + 1e-8)
  influence_norm = (influence - min) / (max - min)   [global min/max]
  noise_std = 0.01 + 0.49 * influence_norm
  noised = f + noise * noise_std      (noise = jax.random.normal(key(1)))

Sharding: data-parallel over queries; batch image i -> core i (784 rows each).
Memory bank replicated. Only the global min/max of influence is all-reduced.

Device algorithm per core:
  score[n,m] = 2 f.mb - |mb|^2: f32r matmul (full TensorE rate; operands
  rounded to 11 mantissa bits by HW) accumulated over 8 k-blocks in PSUM;
  |mb|^2 subtracted in f32 on VectorE during the PSUM->SBUF move.
  argmin_m d2 = argmax_m score: DVE MAX8/MAX_INDEX over 1024-wide m-groups,
  tiny cross-group combine (first-index tie-break like jax top_k). Nearest
  rows fetched with an indirect (gather) DMA; influence on DVE/ACT; the
  per-block post-processing is interleaved into the last m-group's loop so
  it hides under the remaining matmuls. Global min/max via a packed [1,2]
  AllReduce(max) of (max, -min); noising fused with scalar_tensor_tensor.
"""

import numpy as np

B, C, H, W = 8, 1024, 28, 28
M = 16384
N_CORES = 8
NQ = H * W          # 784 queries per core (batch-sharded)
CE = 1024           # contraction dim (= C)
KB = CE // 128      # 8 k-blocks
G = 16              # m-groups
GW = M // G         # 1024 group width
NT = GW // 512      # psum tiles per group
P = 128
NB = (NQ + P - 1) // P          # 7 row blocks (6x128 + 16)
BLK = [(i * P, min(P, NQ - i * P)) for i in range(NB)]
EPS = 1e-8
NOISE_MIN, NOISE_MAX = 0.01, 0.5
BIG = 3.0e7

_CACHE = {}


def build_bass(stage=4):
    """Build the SPMD Bass graph (same program for all 8 cores).

    stage: 3 = core-local min/max (debug), 4 = full (8-core AllReduce).
    """
    if ("nc", stage) in _CACHE:
        return _CACHE[("nc", stage)]
    from contextlib import ExitStack
    import concourse.bass as bass
    import concourse.tile as tile
    from concourse import bacc, mybir

    f32 = mybir.dt.float32
    f32r = mybir.dt.float32r
    u32 = mybir.dt.uint32
    AX = mybir.AxisListType
    OP = mybir.AluOpType
    ACTF = mybir.ActivationFunctionType

    nc = bacc.Bacc("TRN2", target_bir_lowering=False, debug=False,
                   num_devices=N_CORES)

    ft = nc.declare_dram_parameter("ft", [CE, NQ], f32r, isOutput=False)
    mbt = nc.declare_dram_parameter("mbt", [CE, M], f32r, isOutput=False)
    mnorm = nc.declare_dram_parameter("mnorm", [1, M], f32, isOutput=False)
    fq = nc.declare_dram_parameter("fq", [NQ, C], f32, isOutput=False)
    mbrows = nc.declare_dram_parameter("mbrows", [M, C], f32, isOutput=False)
    noise = nc.declare_dram_parameter("noise", [NQ, C], f32, isOutput=False)
    out_x = nc.declare_dram_parameter("out_noised", [NQ, C], f32, isOutput=True)
    out_i = nc.declare_dram_parameter("out_inf", [NQ, 1], f32, isOutput=True)
    out_s = nc.declare_dram_parameter("out_std", [NQ, 1], f32, isOutput=True)

    with tile.TileContext(nc) as tc, ExitStack() as ctx:
        const = ctx.enter_context(tc.tile_pool(name="const", bufs=1))
        mbtp = ctx.enter_context(tc.tile_pool(name="mbtp", bufs=2))
        mnp = ctx.enter_context(tc.tile_pool(name="mnp", bufs=2))
        scp = ctx.enter_context(tc.tile_pool(name="scores", bufs=3))
        psp = ctx.enter_context(tc.tile_pool(name="psum", bufs=4, space="PSUM"))
        wk = ctx.enter_context(tc.tile_pool(name="work", bufs=2))
        sm = ctx.enter_context(tc.tile_pool(name="small", bufs=2))
        drp = ctx.enter_context(tc.tile_pool(name="dram", bufs=1, space="DRAM"))

        # ---- resident tiles ----
        ft_sb = const.tile([P, KB, NQ], f32r)
        nc.sync.dma_start(ft_sb[:], ft.rearrange("(kb p) n -> p kb n", p=P))

        gval = const.tile([P, NB, G, 8], f32)   # top-8 scores per (block, group)
        gidx = const.tile([P, NB, G, 8], u32)   # their in-group indices

        goff = const.tile([P, G], f32)          # column g -> g*GW
        for g in range(G):
            nc.vector.memset(goff[:, g:g + 1], float(g * GW))
        bigc = const.tile([P, G], f32)
        nc.vector.memset(bigc[:], BIG)
        epsc = const.tile([P, 1], f32)
        nc.vector.memset(epsc[:], EPS)

        inf_cols = const.tile([P, NB], f32)     # influence per block column
        infmin = const.tile([P, NB], f32)       # +BIG filled (for min)
        infmax = const.tile([P, NB], f32)       # -BIG filled (for max)
        nc.vector.memset(infmin[:], BIG)
        nc.vector.memset(infmax[:], -BIG)

        # queries + noise resident: prefetched while matmuls run
        fqr = const.tile([P, NB, C], f32)
        nzr = const.tile([P, NB, C], f32)
        qn_cols = const.tile([P, NB], f32)      # |f|^2 per block column
        scr = const.tile([P, C], f32)           # Square scratch (never read)
        for b, (n0, pb) in enumerate(BLK):
            nc.sync.dma_start(fqr[:pb, b, :], fq[n0:n0 + pb, :])
            nc.sync.dma_start(nzr[:pb, b, :], noise[n0:n0 + pb, :])
        for b, (n0, pb) in enumerate(BLK):
            nc.scalar.activation(scr[:pb], fqr[:pb, b, :], ACTF.Square,
                                 accum_out=qn_cols[:pb, b:b + 1])

        def emit_block_post(b, n0, pb):
            """Combine groups -> global argmax -> gather -> influence."""
            vals = gval[:pb, b, :, 0]            # [pb, G] stride-8
            maxv = sm.tile([P, 1], f32, name=f"maxv{b}", tag="maxv")
            nc.vector.tensor_reduce(maxv[:pb], vals, axis=AX.X, op=OP.max)
            mask = sm.tile([P, G], u32, name=f"mask{b}", tag="mask")
            nc.vector.tensor_scalar(mask[:pb], vals, maxv[:pb], None,
                                    op0=OP.is_ge)
            idxf = sm.tile([P, G], f32, name=f"idxf{b}", tag="idxf")
            nc.vector.tensor_copy(idxf[:pb], gidx[:pb, b, :, 0])  # u32 -> f32
            nc.vector.tensor_tensor(idxf[:pb], idxf[:pb], goff[:pb], op=OP.add)
            cand = sm.tile([P, G], f32, name=f"cand{b}", tag="cand")
            nc.vector.select(cand[:pb], mask[:pb], idxf[:pb], bigc[:pb])
            idxm = sm.tile([P, 1], f32, name=f"idxm{b}", tag="idxm")
            nc.vector.tensor_reduce(idxm[:pb], cand[:pb], axis=AX.X, op=OP.min)
            idxu = sm.tile([P, 1], u32, name=f"idxu{b}", tag="idxu")
            nc.vector.tensor_copy(idxu[:pb], idxm[:pb])           # f32 -> u32

            nn = wk.tile([P, C], f32, name=f"nn{b}", tag="nn")
            nc.gpsimd.indirect_dma_start(
                nn[:pb], None, mbrows[:],
                bass.IndirectOffsetOnAxis(ap=idxu[:pb], axis=0),
            )
            fqb = fqr[:, b, :]
            diff = wk.tile([P, C], f32, name=f"diff{b}", tag="diff")
            nc.vector.tensor_tensor(diff[:pb], fqb[:pb], nn[:pb],
                                    op=OP.subtract)
            asum = sm.tile([P, 1], f32, name=f"asum{b}", tag="asum")
            nc.vector.tensor_reduce(asum[:pb], diff[:pb], axis=AX.X, op=OP.add,
                                    apply_absolute_value=True)
            d2 = sm.tile([P, 1], f32, name=f"d2{b}", tag="d2")
            nc.vector.tensor_tensor(d2[:pb], qn_cols[:pb, b:b + 1], maxv[:pb],
                                    op=OP.subtract)
            nc.vector.tensor_scalar_max(d2[:pb], d2[:pb], 0.0)
            dist = sm.tile([P, 1], f32, name=f"dist{b}", tag="dist")
            nc.scalar.activation(dist[:pb], d2[:pb], ACTF.Sqrt, bias=epsc[:pb])
            den = sm.tile([P, 1], f32, name=f"den{b}", tag="den")
            nc.vector.tensor_scalar_add(den[:pb], dist[:pb], EPS)
            rden = sm.tile([P, 1], f32, name=f"rden{b}", tag="rden")
            nc.vector.reciprocal(rden[:pb], den[:pb])
            infb = sm.tile([P, 1], f32, name=f"infb{b}", tag="infb")
            nc.vector.tensor_tensor(infb[:pb], asum[:pb], rden[:pb],
                                    op=OP.mult)
            nc.vector.tensor_scalar_mul(infb[:pb], infb[:pb], 1.0 / C)
            nc.vector.tensor_copy(inf_cols[:pb, b:b + 1], infb[:pb])
            nc.vector.tensor_copy(infmin[:pb, b:b + 1], infb[:pb])
            nc.vector.tensor_copy(infmax[:pb, b:b + 1], infb[:pb])

        # ---- main loop: scores + per-group argmax ----
        # Block post-processing is emitted right after each block's LAST
        # group so it overlaps with the remaining blocks' matmuls.
        for g in range(G):
            mbt_g = mbtp.tile([P, KB, GW], f32r)
            nc.sync.dma_start(
                mbt_g[:],
                mbt[:, g * GW:(g + 1) * GW].rearrange("(kb p) m -> p kb m", p=P),
            )
            mnb = mnp.tile([P, GW], f32, tag="mnb")
            nc.sync.dma_start(
                mnb[:], mnorm[0:1, g * GW:(g + 1) * GW].partition_broadcast(P)
            )
            for b, (n0, pb) in enumerate(BLK):
                sc = scp.tile([P, GW], f32)
                ps = psp.tile([P, GW], f32)     # two PSUM banks
                for t in range(NT):
                    for kb in range(KB):
                        nc.tensor.matmul(
                            ps[:pb, t * 512:(t + 1) * 512],
                            ft_sb[:, kb, n0:n0 + pb],
                            mbt_g[:, kb, t * 512:(t + 1) * 512],
                            start=(kb == 0),
                            stop=(kb == KB - 1),
                        )
                # score = 2 f.mb - mnorm, fused with the PSUM->SBUF move
                nc.vector.tensor_tensor(sc[:pb], ps[:pb], mnb[:pb],
                                        op=OP.subtract)
                nc.vector.max(gval[:pb, b, g, :], sc[:pb, :])
                nc.vector.max_index(gidx[:pb, b, g, :], gval[:pb, b, g, :],
                                    sc[:pb, :])
                if g == G - 1:
                    emit_block_post(b, n0, pb)

        # ---- global min/max: partition-reduce via DRAM, AllReduce ----
        vmax = sm.tile([P, 1], f32)
        nc.vector.tensor_reduce(vmax[:], infmax[:], axis=AX.X, op=OP.max)
        vmin = sm.tile([P, 1], f32)
        nc.vector.tensor_reduce(vmin[:], infmin[:], axis=AX.X, op=OP.min)
        pk = sm.tile([P, 2], f32)
        nc.vector.tensor_copy(pk[:, 0:1], vmax[:])
        nc.vector.tensor_scalar_mul(pk[:, 1:2], vmin[:], -1.0)  # -min

        dr_pack = drp.tile([1, 2 * P], f32)
        nc.sync.dma_start(dr_pack[:].rearrange("o (p c) -> (o p) c", c=2),
                          pk[:])
        flat = sm.tile([1, 2 * P], f32)
        nc.sync.dma_start(flat[:], dr_pack[:])
        red2 = sm.tile([1, 2], f32)
        nc.vector.tensor_reduce(
            red2[:], flat[0:1].rearrange("o (p c) -> o c p", c=2),
            axis=AX.X, op=OP.max,
        )
        g2 = sm.tile([P, 2], f32)
        if stage >= 4:
            cc_in = drp.tile([1, 2], f32)
            nc.sync.dma_start(cc_in[:], red2[:])
            cc_out = drp.tile([1, 2], f32, addr_space="Shared")
            nc.gpsimd.collective_compute(
                "AllReduce", OP.max,
                replica_groups=[list(range(N_CORES))],
                ins=[cc_in[:].opt()],
                outs=[cc_out[:].opt()],
            )
            nc.sync.dma_start(g2[:], cc_out[:].partition_broadcast(P))
        else:
            # local-only min/max (debug)
            dr2 = drp.tile([1, 2], f32)
            nc.sync.dma_start(dr2[:], red2[:])
            nc.sync.dma_start(g2[:], dr2[:].partition_broadcast(P))

        # scale = 1/(max-min) when max-min > EPS else 0
        delta = sm.tile([P, 1], f32)
        nc.vector.tensor_tensor(delta[:], g2[:, 0:1], g2[:, 1:2], op=OP.add)
        maskd = sm.tile([P, 1], f32)
        nc.vector.tensor_scalar(maskd[:], delta[:], EPS, None, op0=OP.is_gt)
        deltac = sm.tile([P, 1], f32)
        nc.vector.tensor_scalar_max(deltac[:], delta[:], EPS)
        sca = sm.tile([P, 1], f32)
        nc.vector.reciprocal(sca[:], deltac[:])
        nc.vector.tensor_tensor(sca[:], sca[:], maskd[:], op=OP.mult)

        # ---- normalize, noise-scale, output ----
        for b, (n0, pb) in enumerate(BLK):
            normb = sm.tile([P, 1], f32, name=f"normb{b}", tag="normb")
            # (inf + (-min)) * scale   [g2[:,1:2] holds -min]
            nc.vector.scalar_tensor_tensor(
                normb[:pb], inf_cols[:pb, b:b + 1], g2[:pb, 1:2], sca[:pb],
                op0=OP.add, op1=OP.mult,
            )
            stdb = sm.tile([P, 1], f32, name=f"stdb{b}", tag="stdb")
            nc.vector.tensor_scalar(stdb[:pb], normb[:pb],
                                    NOISE_MAX - NOISE_MIN, NOISE_MIN,
                                    op0=OP.mult, op1=OP.add)
            ox = wk.tile([P, C], f32, name=f"ox{b}", tag="ox")
            # noise*std + f
            nc.vector.scalar_tensor_tensor(
                ox[:pb], nzr[:, b, :][:pb], stdb[:pb], fqr[:, b, :][:pb],
                op0=OP.mult, op1=OP.add,
            )
            nc.sync.dma_start(out_x[n0:n0 + pb, :], ox[:pb])
            nc.sync.dma_start(out_i[n0:n0 + pb, :], normb[:pb])
            nc.sync.dma_start(out_s[n0:n0 + pb, :], stdb[:pb])

    nc.compile()
    _CACHE[("nc", stage)] = nc
    return nc


def prepare_in_maps(features, memory_bank, noise_all):
    features = np.ascontiguousarray(features, dtype=np.float32)
    memory_bank = np.ascontiguousarray(memory_bank, dtype=np.float32)
    f_t = features.reshape(B, C, NQ)            # per-image f^T (C, HW)

    mbt_np = np.ascontiguousarray(memory_bank.T)
    mnorm_np = (memory_bank * memory_bank).sum(axis=1, dtype=np.float32)[None, :]

    in_maps = []
    for c in range(N_CORES):
        in_maps.append({
            "ft": np.ascontiguousarray(2.0 * f_t[c]),
            "mbt": mbt_np,
            "mnorm": mnorm_np,
            "fq": np.ascontiguousarray(f_t[c].T),
            "mbrows": memory_bank,
            "noise": np.ascontiguousarray(noise_all[c * NQ:(c + 1) * NQ]),
        })
    return in_maps


def assemble_outputs(results):
    noised = np.empty((B, C, H, W), dtype=np.float32)
    inf_norm = np.empty((B, H, W), dtype=np.float32)
    noise_std = np.empty((B, H, W), dtype=np.float32)
    for c in range(N_CORES):
        r = results[c]
        noised[c] = r["out_noised"].reshape(H, W, C).transpose(2, 0, 1)
        inf_norm[c] = r["out_inf"].reshape(H, W)
        noise_std[c] = r["out_std"].reshape(H, W)
    return noised, inf_norm, noise_std


def make_noise():
    """Same expression as the reference; same process/config -> same values."""
    import jax
    import jax.numpy as jnp
    return np.asarray(
        jax.random.normal(jax.random.key(1), (B * H * W, C), dtype=jnp.float32)
    )


def kernel(features, memory_bank):
    from concourse.bass_utils import run_bass_kernel_spmd

    nc = build_bass()
    in_maps = prepare_in_maps(features, memory_bank, make_noise())
    res = run_bass_kernel_spmd(nc, in_maps, list(range(N_CORES)))
    return assemble_outputs(res.results)
